# revision 16
# baseline (speedup 1.0000x reference)
"""Trainium2 kernel for nn_CifarModel: blockwise 48x48 linear (stage 2) +
3x(conv3x3-relu-maxpool2) + FC + log_softmax, data-parallel over 8 cores.

v2 design. Per core (1024 images, 64 groups of GB=16, batches of 8 groups):
 - weights are packed/padded on HOST into device-ready lhsT layouts
 - input transpose: PE transposes [128,96]->[96,128] over 128-image batches
 - stage2: per-(jb,ir) zero-padded [96,48] lhsT matmuls (K embedded, no XB)
 - spreader: per-r selection matmuls put S2 rows at quadrant-aligned 32jc+c,
   so the image scatter becomes 16 aligned 4x-mode DVE copies per group
 - conv1: 4x block-diagonal K=27 M=128 single-tap matmuls on a compact
   108-row replica tile (x-blocks of 8 interleaved into partitions)
 - conv2: K=96 (dx folded) x 3 dy taps, X2=18 trimmed layout
 - conv3: K=128 (dx pair folded) + K=64 remainder, X3=10 trimmed
 - all PSUM goes through a 4-buffer ring of [128,512] bank tiles
Falls back to a host JAX implementation if the Bass path fails.
"""

import numpy as np

N_CORES = 8
B_FULL = 8192
B_CORE = B_FULL // N_CORES  # 1024
GB = 16              # images per group
NG = B_CORE // GB    # 64 groups
BATCH = 128          # images per transpose/stage2 batch
NB = B_CORE // BATCH  # 8 batches
GPB = BATCH // GB    # 8 groups per batch


def _jax_reference(x, W_lin, conv1_w, conv1_b, conv2_w, conv2_b, conv3_w,
                   conv3_b, fc_w, fc_b, stage):
    import jax, jax.numpy as jnp
    from jax import lax
    KEY, CH = 4, 3

    def _conv(x, w, b):
        y = lax.conv_general_dilated(x, w, (1, 1), 'SAME',
                                     dimension_numbers=('NCHW', 'OIHW', 'NCHW'))
        return y + b[None, :, None, None]

    def _maxpool2(x):
        return lax.reduce_window(x, -jnp.inf, lax.max,
                                 (1, 1, 2, 2), (1, 1, 2, 2), 'VALID')

    x = jnp.asarray(np.asarray(x, np.float32).reshape(-1, 32, 32, 3))
    B = x.shape[0]
    if int(stage) == 2:
        xb = x.reshape(B, 8, KEY, 8, KEY, CH)
        xb = xb.transpose(0, 1, 3, 2, 4, 5).reshape(B, 64, 48)
        y = jnp.einsum('bnk,ok->bno', xb, jnp.asarray(np.asarray(W_lin, np.float32)))
        y = y.reshape(B, 8, 8, KEY, KEY, CH).transpose(0, 1, 3, 2, 4, 5)
        x_final = y.reshape(B, 32, 32, 3).transpose(0, 3, 1, 2)
    else:
        x_final = x.transpose(0, 3, 1, 2)
    w1 = jnp.asarray(np.asarray(conv1_w, np.float32).reshape(32, 3, 3, 3))
    w2 = jnp.asarray(np.asarray(conv2_w, np.float32).reshape(64, 32, 3, 3))
    w3 = jnp.asarray(np.asarray(conv3_w, np.float32).reshape(128, 64, 3, 3))
    h = _maxpool2(jax.nn.relu(_conv(x_final, w1, jnp.asarray(conv1_b))))
    h = _maxpool2(jax.nn.relu(_conv(h, w2, jnp.asarray(conv2_b))))
    h = _maxpool2(jax.nn.relu(_conv(h, w3, jnp.asarray(conv3_b))))
    h = h.reshape(-1, 2048)
    logits = h @ jnp.asarray(np.asarray(fc_w, np.float32)).T + jnp.asarray(fc_b)
    return np.asarray(jax.nn.log_softmax(logits, axis=-1), dtype=np.float32)


# ---------------- geometry ----------------
Y1, X1 = 34, 36
RW1 = X1 * GB          # 576
L1 = Y1 * X1 * GB      # 19584
Y2, X2 = 18, 18
RW2 = X2 * GB          # 288
L2 = Y2 * X2 * GB      # 5184
Y3, X3 = 10, 10
RW3 = X3 * GB          # 160
L3 = Y3 * X3 * GB      # 1600
M3 = 256               # conv3 front/back margin


def _pack_weights(W_lin, conv1_w, conv2_w, conv3_w, fc_w,
                  conv1_b, conv2_b, conv3_b, fc_b):
    """Host-side packing into device-ready (transposed/padded) layouts."""
    f = np.float32
    W_lin = np.asarray(W_lin, f)               # [48,48]
    w1 = np.asarray(conv1_w, f).reshape(32, 3, 3, 3)
    w2 = np.asarray(conv2_w, f).reshape(64, 32, 3, 3)
    w3 = np.asarray(conv3_w, f).reshape(128, 64, 3, 3)
    fc = np.asarray(fc_w, f)                   # [10,2048]

    # stage2 lhsT per (jb, ir): [96,48], rows 12jb+q hold W_lin[:, 12ir+q].T
    s2w = np.zeros((32, 96, 48), f)
    for jb in range(8):
        for ir in range(4):
            s2w[jb * 4 + ir, 12 * jb:12 * jb + 12, :] = \
                W_lin[:, 12 * ir:12 * ir + 12].T
    # spreader selection: sel[r][12r+3jc+c, 32jc+c] = 1
    sel = np.zeros((4, 48, 128), f)
    for r in range(4):
        for jc in range(4):
            for c in range(3):
                sel[r, 12 * r + 3 * jc + c, 32 * jc + c] = 1.0
    # conv1 block-diag: rows 32p + (9dyb+3dxb+c), cols 32p + cout
    # dy(dyb), dx(dxb) = (1,0,2)
    ord3 = (1, 0, 2)
    w1bd = np.zeros((128, 128), f)
    for p in range(4):
        for dyb in range(3):
            for dxb in range(3):
                for c in range(3):
                    w1bd[32 * p + 9 * dyb + 3 * dxb + c, 32 * p:32 * p + 32] = \
                        w1[:, c, ord3[dyb], ord3[dxb]]
    # conv2: w2t[dy][32kx+c, cout]
    w2t = np.zeros((3, 96, 64), f)
    for dy in range(3):
        for kx in range(3):
            for c in range(32):
                w2t[dy, 32 * kx + c, :] = w2[:, c, dy, kx]
    # conv3 folded (kx 0,1) + remainder (kx 2)
    w3f = np.zeros((3, 128, 128), f)
    w3r = np.zeros((3, 128, 128), f)
    for dy in range(3):
        for q in range(2):
            for c in range(64):
                w3f[dy, 64 * q + c, :] = w3[:, c, dy, q]
        for c in range(64):
            w3r[dy, 64 + c, :] = w3[:, c, dy, 2]
    # fc taps: fct[k][c, o] = fc_w[o, 16c+k]
    fct = np.zeros((16, 128, 10), f)
    for k in range(16):
        fct[k] = fc[:, k::16].T
    b1x4 = np.tile(np.asarray(conv1_b, f), 4)          # [128]
    return {
        "s2w": s2w.reshape(32 * 96, 48),
        "selw": sel.reshape(4 * 48, 128),
        "w1bd": w1bd,
        "w2t": w2t.reshape(3 * 96, 64),
        "w3f": w3f.reshape(3 * 128, 128),
        "w3r": w3r.reshape(3 * 128, 128),
        "fct": fct.reshape(16 * 128, 10),
        "b1x4": b1x4.reshape(128, 1),
        "b2": np.asarray(conv2_b, f).reshape(64, 1),
        "b3": np.asarray(conv3_b, f).reshape(128, 1),
        "fcb": np.asarray(fc_b, f).reshape(10, 1),
    }


def _build_bass():
    import concourse.bass as bass
    import concourse.bacc as bacc
    import concourse.tile as tile
    from concourse import mybir
    from concourse.masks import make_identity

    f32 = mybir.dt.float32
    bf16 = mybir.dt.bfloat16
    AP = bass.AP
    OP = mybir.AluOpType
    ACTF = mybir.ActivationFunctionType

    nc = bacc.Bacc("TRN2", target_bir_lowering=False, debug=False,
                   num_devices=N_CORES)
    x_d = nc.dram_tensor("x", [B_CORE, 3072], f32, kind="ExternalInput")
    s2w_d = nc.dram_tensor("s2w", [32 * 96, 48], f32, kind="ExternalInput")
    sel_d = nc.dram_tensor("selw", [4 * 48, 128], f32, kind="ExternalInput")
    w1_d = nc.dram_tensor("w1bd", [128, 128], f32, kind="ExternalInput")
    w2_d = nc.dram_tensor("w2t", [3 * 96, 64], f32, kind="ExternalInput")
    w3f_d = nc.dram_tensor("w3f", [3 * 128, 128], f32, kind="ExternalInput")
    w3r_d = nc.dram_tensor("w3r", [3 * 128, 128], f32, kind="ExternalInput")
    fct_d = nc.dram_tensor("fct", [16 * 128, 10], f32, kind="ExternalInput")
    b1_d = nc.dram_tensor("b1x4", [128, 1], f32, kind="ExternalInput")
    b2_d = nc.dram_tensor("b2", [64, 1], f32, kind="ExternalInput")
    b3_d = nc.dram_tensor("b3", [128, 1], f32, kind="ExternalInput")
    fcb_d = nc.dram_tensor("fcb", [10, 1], f32, kind="ExternalInput")
    out_d = nc.dram_tensor("out", [B_CORE, 10], f32, kind="ExternalOutput")

    def pget(t):
        a = t[:, :]
        return a.tensor, a.offset, a.ap[0][0]

    with tile.TileContext(nc) as tc:
        with tc.tile_pool(name="persist", bufs=1) as P, \
             tc.tile_pool(name="work", bufs=1) as W, \
             tc.tile_pool(name="stage", bufs=2) as SG, \
             tc.tile_pool(name="psum", bufs=2, space="PSUM") as PS, \
             tc.tile_pool(name="psumB", bufs=3, space="PSUM") as PB:

            ident = P.tile([128, 128], f32)
            make_identity(nc, ident)

            def ring():
                return PS.tile([128, 512], f32, tag="ring", name="ringt")

            def big():
                return PB.tile([128, 1024], f32, tag="big", name="bigt")

            # ---- weight load + cast (setup only) ----
            def wload(dram_t, rows, cols, nslices, tag):
                o = P.tile([rows, nslices * cols], bf16, tag=tag, name=f"w_{tag}")
                # layout: slice s occupies cols [s*cols, (s+1)*cols)
                for s in range(nslices):
                    st = SG.tile([rows, cols], f32, tag=f"wst_{tag}", name=f"wst_{tag}")
                    nc.sync.dma_start(
                        out=st[:, :],
                        in_=dram_t[s * rows:(s + 1) * rows, :])
                    nc.vector.tensor_copy(
                        out=o[:, s * cols:(s + 1) * cols], in_=st[:, :])
                return o

            S2W = wload(s2w_d, 96, 48, 32, "S2W")     # [96, 32*48]
            SEL = wload(sel_d, 48, 128, 4, "SEL")     # [48, 4*128]
            W1BD = wload(w1_d, 128, 128, 1, "W1BD")
            W2T = wload(w2_d, 96, 64, 3, "W2T")
            W3F = wload(w3f_d, 128, 128, 3, "W3F")
            W3R = wload(w3r_d, 128, 128, 3, "W3R")
            FCT = wload(fct_d, 128, 10, 16, "FCT")

            def bload(dram_t, rows, tag):
                b = P.tile([rows, 1], f32, tag=tag, name=f"b_{tag}")
                nc.sync.dma_start(out=b[:, :], in_=dram_t[:, :])
                return b
            B1 = bload(b1_d, 128, "B1")
            B2 = bload(b2_d, 64, "B2")
            B3 = bload(b3_d, 128, "B3")
            FCB = bload(fcb_d, 10, "FCB")

            # ---- persistent activations ----
            A1C = P.tile([27, L1], bf16)           # padded replica-27 tile
            A1R = P.tile([128, 32 * 8 * GB], bf16)  # compact 108-row conv1 rhs
            A2 = P.tile([96, L2], bf16)
            A3 = P.tile([128, M3 + L3 + M3], bf16)
            A4 = P.tile([128, NG * 16 * GB], bf16)
            nc.vector.memset(A1C[:, :], 0.0)
            nc.vector.memset(A1R[:, :], 0.0)
            nc.vector.memset(A2[:, :], 0.0)
            nc.vector.memset(A3[:, :], 0.0)

            a1c_t, a1c_o, a1c_p = pget(A1C)
            a1r_t, a1r_o, a1r_p = pget(A1R)
            a2_t, a2_o, a2_p = pget(A2)
            a3_t, a3_o, a3_p = pget(A3)
            a4_t, a4_o, a4_p = pget(A4)

            def chunked(n, step=512):
                base = 0
                while base < n:
                    yield base, min(step, n - base)
                    base += step

            def batch_front(bi):
                """Load 128 images; transpose; stage2; spread to S2R."""
                xr = W.tile([128, 3072], f32, tag="xr")
                nc.sync.dma_start(out=xr[:, :],
                                  in_=x_d[bi * BATCH:(bi + 1) * BATCH, :])
                XT = W.tile([96, 32 * 128], bf16, tag="XT")
                for j in range(8):   # 4 transposes -> one ring tile
                    pt = ring()
                    for t in range(4):
                        y = 4 * j + t
                        nc.tensor.transpose(
                            pt[:96, t * 128:(t + 1) * 128],
                            xr[:, 96 * y:96 * (y + 1)], ident[:, :])
                    nc.vector.tensor_copy(
                        out=XT[:, j * 512:(j + 1) * 512], in_=pt[:96, :])
                xt_t, xt_o, xt_p = pget(XT)

                # stage2: S2 [48, (jb, ib, img128)]
                S2 = W.tile([48, 8 * 8 * 128], bf16, tag="S2")
                s2_t, s2_o, s2_p = pget(S2)
                for jb in range(8):
                    for h in range(2):  # img halves of 64
                        ps = ring()
                        for ir in range(4):
                            rhs = AP(tensor=xt_t,
                                     offset=xt_o + ir * 128 + h * 64,
                                     ap=[[xt_p, 96], [512, 8], [1, 64]])
                            nc.tensor.matmul(
                                ps[:48, :], S2W[:, (jb * 4 + ir) * 48:
                                                (jb * 4 + ir + 1) * 48],
                                rhs, start=(ir == 0), stop=(ir == 3))
                        nc.vector.tensor_copy(
                            out=AP(tensor=s2_t,
                                   offset=s2_o + jb * 1024 + h * 64,
                                   ap=[[s2_p, 48], [128, 8], [1, 64]]),
                            in_=ps[:48, :])
                return S2

            def spread(S2, gp):
                """Spreader for group-pair gp: S2R [128,(r,jb,ib,img32)]."""
                s2_t, s2_o, s2_p = pget(S2)
                S2R = W.tile([128, 4 * 2048], bf16, tag="S2R")
                s2r_t, s2r_o, s2r_p = pget(S2R)
                for r in range(4):
                    for q in range(4):  # jb pairs of 2 -> N=512
                        ps = ring()
                        rhs = AP(tensor=s2_t,
                                 offset=s2_o + q * 2048 + gp * 32,
                                 ap=[[s2_p, 48], [1024, 2], [128, 8],
                                     [1, 32]])
                        nc.tensor.matmul(ps[:, :],
                                         SEL[:, r * 128:(r + 1) * 128],
                                         rhs, start=True, stop=True)
                        nc.scalar.activation(
                            out=S2R[:, r * 2048 + q * 512:
                                    r * 2048 + (q + 1) * 512],
                            in_=ps[:, :], func=ACTF.Copy, scale=1.0)
                return S2R

            def scatter(S2R, gl):
                """Scatter one group's center into A1C rows 0..3."""
                s2r_t, s2r_o, s2r_p = pget(S2R)
                for r in range(4):
                    for jc in range(4):
                        src = AP(tensor=s2r_t,
                                 offset=s2r_o + 32 * jc * s2r_p + r * 2048
                                 + (gl & 1) * 16,
                                 ap=[[s2r_p, 3], [256, 8], [32, 8], [1, 16]])
                        dst = AP(tensor=a1c_t,
                                 offset=a1c_o
                                 + ((r + 1) * X1 + jc + 1) * GB,
                                 ap=[[a1c_p, 3], [4 * GB, 8],
                                     [4 * RW1, 8], [1, GB]])
                        nc.vector.tensor_copy(out=dst, in_=src)

            def conv_body(g):
                # --- conv1 replicas: A1C rows 3..27 (4 chunked DMAs) ---
                def rep(dst_row, dst_col, src_row, src_col, nrows, length):
                    # split columns so descriptors spread over DMA engines
                    n3 = (length // 3264) or 1
                    main = n3 * 3264 if length >= 3264 else 0
                    if main:
                        nc.sync.dma_start(
                            out=AP(tensor=a1c_t,
                                   offset=a1c_o + dst_row * a1c_p + dst_col,
                                   ap=[[a1c_p, nrows], [3264, n3], [1, 3264]]),
                            in_=AP(tensor=a1c_t,
                                   offset=a1c_o + src_row * a1c_p + src_col,
                                   ap=[[a1c_p, nrows], [3264, n3], [1, 3264]]))
                    rest = length - main
                    if rest:
                        nc.sync.dma_start(
                            out=AP(tensor=a1c_t,
                                   offset=a1c_o + dst_row * a1c_p + dst_col
                                   + main,
                                   ap=[[a1c_p, nrows], [1, rest]]),
                            in_=AP(tensor=a1c_t,
                                   offset=a1c_o + src_row * a1c_p + src_col
                                   + main,
                                   ap=[[a1c_p, nrows], [1, rest]]))
                rep(3, GB, 0, 0, 3, L1 - GB)       # dx=0 block
                rep(6, 0, 0, GB, 3, L1 - GB)       # dx=2 block
                rep(9, RW1, 0, 0, 9, L1 - RW1)     # dy=0 block
                rep(18, 0, 0, RW1, 9, L1 - RW1)    # dy=2 block
                # --- stage II: compact 108-row tile (4 DMAs, one per p) ---
                for p in range(4):
                    nc.sync.dma_start(
                        out=AP(tensor=a1r_t,
                               offset=a1r_o + 32 * p * a1r_p,
                               ap=[[a1r_p, 27], [128, 32], [1, 128]]),
                        in_=AP(tensor=a1c_t,
                               offset=a1c_o + RW1 + (8 * p + 1) * GB,
                               ap=[[a1c_p, 27], [RW1, 32], [1, 128]]))
                # --- conv1: 8 single-tap matmuls N=512, 4 wide act drains ---
                T1 = W.tile([128, 32 * 8 * GB], bf16, tag="T1")
                t1_t, t1_o, t1_p = pget(T1)
                for base, n in chunked(32 * 8 * GB, 1024):
                    ps = big()
                    for sub in range(0, n, 512):
                        rhs = AP(tensor=a1r_t, offset=a1r_o + base + sub,
                                 ap=[[a1r_p, 128], [1, 512]])
                        nc.tensor.matmul(ps[:, sub:sub + 512], W1BD[:, :],
                                         rhs, start=True, stop=True)
                    nc.scalar.activation(out=T1[:, base:base + n],
                                         in_=ps[:, :n], func=ACTF.Relu,
                                         bias=B1[:, :], scale=1.0)
                # --- pool1 (rows (p,c), cols (y32, k8, b)) ---
                X1p = W.tile([128, 32 * 4 * GB], bf16, tag="X1p")
                x1_t, x1_o, x1_p = pget(X1p)
                nc.vector.tensor_tensor(
                    out=X1p[:, :],
                    in0=AP(tensor=t1_t, offset=t1_o,
                           ap=[[t1_p, 128], [128, 32], [32, 4], [1, GB]]),
                    in1=AP(tensor=t1_t, offset=t1_o + GB,
                           ap=[[t1_p, 128], [128, 32], [32, 4], [1, GB]]),
                    op=OP.max)
                X1q = W.tile([128, 16 * 4 * GB], bf16, tag="X1q")
                x1q_t, x1q_o, x1q_p = pget(X1q)
                nc.vector.tensor_tensor(
                    out=X1q[:, :],
                    in0=AP(tensor=x1_t, offset=x1_o,
                           ap=[[x1_p, 128], [128, 16], [1, 64]]),
                    in1=AP(tensor=x1_t, offset=x1_o + 64,
                           ap=[[x1_p, 128], [128, 16], [1, 64]]),
                    op=OP.max)
                # regroup 4 x-blocks into A2 center rows 32..64
                for p in range(4):
                    nc.vector.tensor_copy(
                        out=AP(tensor=a2_t,
                               offset=a2_o + 32 * a2_p + RW2
                               + (4 * p + 1) * GB,
                               ap=[[a2_p, 32], [RW2, 16], [1, 64]]),
                        in_=AP(tensor=x1q_t,
                               offset=x1q_o + 32 * p * x1q_p,
                               ap=[[x1q_p, 32], [64, 16], [1, 64]]))
                # --- A2 dx replicas (2 chunked DMAs over L2-GB=5168) ---
                nc.sync.dma_start(
                    out=AP(tensor=a2_t, offset=a2_o + GB,
                           ap=[[a2_p, 32], [1292, 4], [1, 1292]]),
                    in_=AP(tensor=a2_t, offset=a2_o + 32 * a2_p,
                           ap=[[a2_p, 32], [1292, 4], [1, 1292]]))
                nc.sync.dma_start(
                    out=AP(tensor=a2_t, offset=a2_o + 64 * a2_p,
                           ap=[[a2_p, 32], [1292, 4], [1, 1292]]),
                    in_=AP(tensor=a2_t, offset=a2_o + 32 * a2_p + GB,
                           ap=[[a2_p, 32], [1292, 4], [1, 1292]]))
                # --- conv2: 3 dy taps, cols [RW2, 17*RW2) ---
                T2 = W.tile([64, 16 * RW2], bf16, tag="T2")
                t2_t, t2_o, t2_p = pget(T2)
                for base, n in chunked(16 * RW2, 1024):
                    ps = big()
                    for sub in range(0, n, 512):
                        ns = min(512, n - sub)
                        for dy in range(3):
                            rhs = AP(tensor=a2_t,
                                     offset=a2_o + base + sub + dy * RW2,
                                     ap=[[a2_p, 96], [1, ns]])
                            nc.tensor.matmul(ps[:64, sub:sub + ns],
                                             W2T[:, dy * 64:(dy + 1) * 64],
                                             rhs, start=(dy == 0),
                                             stop=(dy == 2))
                    nc.scalar.activation(out=T2[:, base:base + n],
                                         in_=ps[:64, :n], func=ACTF.Relu,
                                         bias=B2[:, :], scale=1.0)
                # --- pool2 -> A3 center rows 64..128 ---
                X2p = W.tile([64, 16 * 8 * GB], bf16, tag="X2p")
                x2_t, x2_o, x2_p = pget(X2p)
                nc.vector.tensor_tensor(
                    out=X2p[:, :],
                    in0=AP(tensor=t2_t, offset=t2_o + GB,
                           ap=[[t2_p, 64], [RW2, 16], [32, 8], [1, GB]]),
                    in1=AP(tensor=t2_t, offset=t2_o + 2 * GB,
                           ap=[[t2_p, 64], [RW2, 16], [32, 8], [1, GB]]),
                    op=OP.max)
                nc.vector.tensor_tensor(
                    out=AP(tensor=a3_t,
                           offset=a3_o + 64 * a3_p + M3 + RW3 + GB,
                           ap=[[a3_p, 64], [RW3, 8], [1, 128]]),
                    in0=AP(tensor=x2_t, offset=x2_o,
                           ap=[[x2_p, 64], [256, 8], [1, 128]]),
                    in1=AP(tensor=x2_t, offset=x2_o + 128,
                           ap=[[x2_p, 64], [256, 8], [1, 128]]),
                    op=OP.max)
                # --- A3 dx replica: rows 0..64 = center shifted +GB ---
                nc.vector.tensor_copy(
                    out=AP(tensor=a3_t, offset=a3_o + GB,
                           ap=[[a3_p, 64], [1, M3 + L3 + M3 - GB]]),
                    in_=AP(tensor=a3_t, offset=a3_o + 64 * a3_p,
                           ap=[[a3_p, 64], [1, M3 + L3 + M3 - GB]]))
                # --- conv3: (K=128 fold + K=64 rem) x 3 dy ---
                T3 = W.tile([128, 8 * RW3], bf16, tag="T3")
                t3_t, t3_o, t3_p = pget(T3)
                for base, n in chunked(8 * RW3, 1024):
                    ps = big()
                    for sub in range(0, n, 512):
                        ns = min(512, n - sub)
                        for dy in range(3):
                            rhs = AP(tensor=a3_t,
                                     offset=a3_o + M3 + base + sub
                                     + dy * RW3,
                                     ap=[[a3_p, 128], [1, ns]])
                            nc.tensor.matmul(ps[:, sub:sub + ns],
                                             W3F[:, dy * 128:(dy + 1) * 128],
                                             rhs, start=(dy == 0), stop=False)
                            rhs2 = AP(tensor=a3_t,
                                      offset=a3_o + 64 * a3_p + M3 + base
                                      + sub + dy * RW3 + GB,
                                      ap=[[a3_p, 64], [1, ns]])
                            nc.tensor.matmul(ps[:, sub:sub + ns],
                                             W3R[64:128,
                                                 dy * 128:(dy + 1) * 128],
                                             rhs2, start=False,
                                             stop=(dy == 2))
                    nc.scalar.activation(out=T3[:, base:base + n],
                                         in_=ps[:, :n], func=ACTF.Relu,
                                         bias=B3[:, :], scale=1.0)
                # --- pool3 -> A4 ---
                X3p = W.tile([128, 8 * 4 * GB], bf16, tag="X3p")
                x3_t, x3_o, x3_p = pget(X3p)
                nc.vector.tensor_tensor(
                    out=X3p[:, :],
                    in0=AP(tensor=t3_t, offset=t3_o + GB,
                           ap=[[t3_p, 128], [RW3, 8], [32, 4], [1, GB]]),
                    in1=AP(tensor=t3_t, offset=t3_o + 2 * GB,
                           ap=[[t3_p, 128], [RW3, 8], [32, 4], [1, GB]]),
                    op=OP.max)
                nc.vector.tensor_tensor(
                    out=A4[:, g * 256:(g + 1) * 256],
                    in0=AP(tensor=x3_t, offset=x3_o,
                           ap=[[x3_p, 128], [128, 4], [1, 64]]),
                    in1=AP(tensor=x3_t, offset=x3_o + 64,
                           ap=[[x3_p, 128], [128, 4], [1, 64]]),
                    op=OP.max)

            for bi in range(NB):
                S2 = batch_front(bi)
                for gp in range(4):
                    S2R = spread(S2, gp)
                    for gh in range(2):
                        gl = gp * 2 + gh
                        g = bi * GPB + gl
                        scatter(S2R, gl)
                        conv_body(g)

            # ---------------- FC + log_softmax ----------------
            for bq in range(2):  # 512 images each
                ps = ring()
                for k in range(16):
                    rhs = AP(tensor=a4_t,
                             offset=a4_o + bq * 32 * 256 + k * GB,
                             ap=[[a4_p, 128], [256, 32], [1, GB]])
                    nc.tensor.matmul(ps[:10, :], FCT[:, k * 10:(k + 1) * 10],
                                     rhs, start=(k == 0), stop=(k == 15))
                lg = W.tile([10, 512], f32, tag="lgs")
                nc.vector.tensor_scalar(lg[:, :], ps[:10, :], FCB[:, :],
                                        None, OP.add)
                for h in range(4):  # 128 images per transpose
                    pt = ring()
                    nc.tensor.transpose(pt[:, :10],
                                        lg[:, h * 128:(h + 1) * 128],
                                        ident[:10, :10])
                    z = W.tile([128, 10], f32, tag="z")
                    nc.vector.tensor_copy(out=z[:, :], in_=pt[:, :10])
                    m = W.tile([128, 1], f32, tag="m")
                    nc.vector.tensor_reduce(out=m[:, :], in_=z[:, :],
                                            axis=mybir.AxisListType.X,
                                            op=OP.max, negate=True)
                    e = W.tile([128, 10], f32, tag="e")
                    nc.scalar.activation(out=e[:, :], in_=z[:, :],
                                         func=ACTF.Exp, bias=m[:, :],
                                         scale=1.0)
                    s = W.tile([128, 1], f32, tag="s")
                    nc.vector.tensor_reduce(out=s[:, :], in_=e[:, :],
                                            axis=mybir.AxisListType.X,
                                            op=OP.add)
                    ls = W.tile([128, 1], f32, tag="ls")
                    nc.scalar.activation(out=ls[:, :], in_=s[:, :],
                                         func=ACTF.Ln)
                    nc.vector.tensor_scalar(ls[:, :], ls[:, :], m[:, :],
                                            None, OP.subtract)
                    o = W.tile([128, 10], f32, tag="o")
                    nc.vector.tensor_scalar(o[:, :], z[:, :], ls[:, :],
                                            None, OP.subtract)
                    bc = bq * 4 + h
                    nc.sync.dma_start(out=out_d[bc * 128:(bc + 1) * 128, :],
                                      in_=o[:, :])

    nc.compile()
    return nc


_NC_CACHE = {}
LAST_RESULT = None
LAST_USED_BASS = False


def _run_bass(x, W_lin, conv1_w, conv1_b, conv2_w, conv2_b, conv3_w, conv3_b,
              fc_w, fc_b, stage, trace=False):
    global LAST_RESULT
    from concourse.bass_utils import run_bass_kernel_spmd
    if int(stage) != 2:
        raise NotImplementedError("bass path only implements stage=2")
    if "nc" not in _NC_CACHE:
        _NC_CACHE["nc"] = _build_bass()
    nc = _NC_CACHE["nc"]
    xs = np.ascontiguousarray(x, dtype=np.float32).reshape(N_CORES, B_CORE, 3072)
    common = _pack_weights(W_lin, conv1_w, conv2_w, conv3_w, fc_w,
                           conv1_b, conv2_b, conv3_b, fc_b)
    in_maps = [dict(common, x=xs[i]) for i in range(N_CORES)]
    res = run_bass_kernel_spmd(nc, in_maps, core_ids=list(range(N_CORES)),
                               trace=trace)
    LAST_RESULT = res
    return np.concatenate([r["out"] for r in res.results], axis=0)


def kernel(**inputs) -> np.ndarray:
    import os
    global LAST_USED_BASS
    stage = inputs.get("stage", 2)
    args = {k: np.asarray(v) for k, v in inputs.items() if k != "stage"}
    trace = os.environ.get("KERNEL_TRACE", "") == "1"
    try:
        out = _run_bass(stage=stage, trace=trace, **args)
        LAST_USED_BASS = True
        return out
    except Exception as e:
        import traceback, sys
        traceback.print_exc()
        print(f"[kernel] Bass path failed ({type(e).__name__}); "
              "falling back to JAX host implementation", file=sys.stderr)
        LAST_USED_BASS = False
        return _jax_reference(stage=stage, **args)


# revision 18
# speedup vs baseline: 1.0285x; 1.0285x over previous
"""Trainium2 kernel for nn_CifarModel: blockwise 48x48 linear (stage 2) +
3x(conv3x3-relu-maxpool2) + FC + log_softmax, data-parallel over 8 cores.

v2 design. Per core (1024 images, 64 groups of GB=16, batches of 8 groups):
 - weights are packed/padded on HOST into device-ready lhsT layouts
 - input transpose: PE transposes [128,96]->[96,128] over 128-image batches
 - stage2: per-(jb,ir) zero-padded [96,48] lhsT matmuls (K embedded, no XB)
 - spreader: per-r selection matmuls put S2 rows at quadrant-aligned 32jc+c,
   so the image scatter becomes 16 aligned 4x-mode DVE copies per group
 - conv1: 4x block-diagonal K=27 M=128 single-tap matmuls on a compact
   108-row replica tile (x-blocks of 8 interleaved into partitions)
 - conv2: K=96 (dx folded) x 3 dy taps, X2=18 trimmed layout
 - conv3: K=128 (dx pair folded) + K=64 remainder, X3=10 trimmed
 - all PSUM goes through a 4-buffer ring of [128,512] bank tiles
Falls back to a host JAX implementation if the Bass path fails.
"""

import numpy as np

N_CORES = 8
B_FULL = 8192
B_CORE = B_FULL // N_CORES  # 1024
GB = 16              # images per group
NG = B_CORE // GB    # 64 groups
BATCH = 128          # images per transpose/stage2 batch
NB = B_CORE // BATCH  # 8 batches
GPB = BATCH // GB    # 8 groups per batch


def _jax_reference(x, W_lin, conv1_w, conv1_b, conv2_w, conv2_b, conv3_w,
                   conv3_b, fc_w, fc_b, stage):
    import jax, jax.numpy as jnp
    from jax import lax
    KEY, CH = 4, 3

    def _conv(x, w, b):
        y = lax.conv_general_dilated(x, w, (1, 1), 'SAME',
                                     dimension_numbers=('NCHW', 'OIHW', 'NCHW'))
        return y + b[None, :, None, None]

    def _maxpool2(x):
        return lax.reduce_window(x, -jnp.inf, lax.max,
                                 (1, 1, 2, 2), (1, 1, 2, 2), 'VALID')

    x = jnp.asarray(np.asarray(x, np.float32).reshape(-1, 32, 32, 3))
    B = x.shape[0]
    if int(stage) == 2:
        xb = x.reshape(B, 8, KEY, 8, KEY, CH)
        xb = xb.transpose(0, 1, 3, 2, 4, 5).reshape(B, 64, 48)
        y = jnp.einsum('bnk,ok->bno', xb, jnp.asarray(np.asarray(W_lin, np.float32)))
        y = y.reshape(B, 8, 8, KEY, KEY, CH).transpose(0, 1, 3, 2, 4, 5)
        x_final = y.reshape(B, 32, 32, 3).transpose(0, 3, 1, 2)
    else:
        x_final = x.transpose(0, 3, 1, 2)
    w1 = jnp.asarray(np.asarray(conv1_w, np.float32).reshape(32, 3, 3, 3))
    w2 = jnp.asarray(np.asarray(conv2_w, np.float32).reshape(64, 32, 3, 3))
    w3 = jnp.asarray(np.asarray(conv3_w, np.float32).reshape(128, 64, 3, 3))
    h = _maxpool2(jax.nn.relu(_conv(x_final, w1, jnp.asarray(conv1_b))))
    h = _maxpool2(jax.nn.relu(_conv(h, w2, jnp.asarray(conv2_b))))
    h = _maxpool2(jax.nn.relu(_conv(h, w3, jnp.asarray(conv3_b))))
    h = h.reshape(-1, 2048)
    logits = h @ jnp.asarray(np.asarray(fc_w, np.float32)).T + jnp.asarray(fc_b)
    return np.asarray(jax.nn.log_softmax(logits, axis=-1), dtype=np.float32)


# ---------------- geometry ----------------
Y1, X1 = 34, 36
RW1 = X1 * GB          # 576
L1 = Y1 * X1 * GB      # 19584
Y2, X2 = 18, 18
RW2 = X2 * GB          # 288
L2 = Y2 * X2 * GB      # 5184
Y3, X3 = 10, 10
RW3 = X3 * GB          # 160
L3 = Y3 * X3 * GB      # 1600
M3 = 256               # conv3 front/back margin


def _pack_weights(W_lin, conv1_w, conv2_w, conv3_w, fc_w,
                  conv1_b, conv2_b, conv3_b, fc_b):
    """Host-side packing into device-ready (transposed/padded) layouts."""
    f = np.float32
    W_lin = np.asarray(W_lin, f)               # [48,48]
    w1 = np.asarray(conv1_w, f).reshape(32, 3, 3, 3)
    w2 = np.asarray(conv2_w, f).reshape(64, 32, 3, 3)
    w3 = np.asarray(conv3_w, f).reshape(128, 64, 3, 3)
    fc = np.asarray(fc_w, f)                   # [10,2048]

    # stage2 lhsT per (jb, ir): [96,48], rows 12jb+q hold W_lin[:, 12ir+q].T
    s2w = np.zeros((32, 96, 48), f)
    for jb in range(8):
        for ir in range(4):
            s2w[jb * 4 + ir, 12 * jb:12 * jb + 12, :] = \
                W_lin[:, 12 * ir:12 * ir + 12].T
    # spreader selection: sel[r][12r+3jc+c, 32jc+c] = 1
    sel = np.zeros((4, 48, 128), f)
    for r in range(4):
        for jc in range(4):
            for c in range(3):
                sel[r, 12 * r + 3 * jc + c, 32 * jc + c] = 1.0
    # conv1 block-diag: rows 32p + (9dyb+3dxb+c), cols 32p + cout
    # dy(dyb), dx(dxb) = (1,0,2)
    ord3 = (1, 0, 2)
    w1bd = np.zeros((128, 128), f)
    for p in range(4):
        for dyb in range(3):
            for dxb in range(3):
                for c in range(3):
                    w1bd[32 * p + 9 * dyb + 3 * dxb + c, 32 * p:32 * p + 32] = \
                        w1[:, c, ord3[dyb], ord3[dxb]]
    # conv2: w2t[dy][32kx+c, cout]
    w2t = np.zeros((3, 96, 64), f)
    for dy in range(3):
        for kx in range(3):
            for c in range(32):
                w2t[dy, 32 * kx + c, :] = w2[:, c, dy, kx]
    # conv3 folded (kx 0,1) + remainder (kx 2)
    w3f = np.zeros((3, 128, 128), f)
    w3r = np.zeros((3, 128, 128), f)
    for dy in range(3):
        for q in range(2):
            for c in range(64):
                w3f[dy, 64 * q + c, :] = w3[:, c, dy, q]
        for c in range(64):
            w3r[dy, 64 + c, :] = w3[:, c, dy, 2]
    # fc taps: fct[k][c, o] = fc_w[o, 16c+k]
    fct = np.zeros((16, 128, 10), f)
    for k in range(16):
        fct[k] = fc[:, k::16].T
    b1x4 = np.tile(np.asarray(conv1_b, f), 4)          # [128]
    return {
        "s2w": s2w.reshape(32 * 96, 48),
        "selw": sel.reshape(4 * 48, 128),
        "w1bd": w1bd,
        "w2t": w2t.reshape(3 * 96, 64),
        "w3f": w3f.reshape(3 * 128, 128),
        "w3r": w3r.reshape(3 * 128, 128),
        "fct": fct.reshape(16 * 128, 10),
        "b1x4": b1x4.reshape(128, 1),
        "b2": np.asarray(conv2_b, f).reshape(64, 1),
        "b3": np.asarray(conv3_b, f).reshape(128, 1),
        "fcb": np.asarray(fc_b, f).reshape(10, 1),
    }


def _build_bass():
    import concourse.bass as bass
    import concourse.bacc as bacc
    import concourse.tile as tile
    from concourse import mybir
    from concourse.masks import make_identity

    f32 = mybir.dt.float32
    bf16 = mybir.dt.bfloat16
    AP = bass.AP
    OP = mybir.AluOpType
    ACTF = mybir.ActivationFunctionType

    nc = bacc.Bacc("TRN2", target_bir_lowering=False, debug=False,
                   num_devices=N_CORES)
    x_d = nc.dram_tensor("x", [B_CORE, 3072], f32, kind="ExternalInput")
    s2w_d = nc.dram_tensor("s2w", [32 * 96, 48], f32, kind="ExternalInput")
    sel_d = nc.dram_tensor("selw", [4 * 48, 128], f32, kind="ExternalInput")
    w1_d = nc.dram_tensor("w1bd", [128, 128], f32, kind="ExternalInput")
    w2_d = nc.dram_tensor("w2t", [3 * 96, 64], f32, kind="ExternalInput")
    w3f_d = nc.dram_tensor("w3f", [3 * 128, 128], f32, kind="ExternalInput")
    w3r_d = nc.dram_tensor("w3r", [3 * 128, 128], f32, kind="ExternalInput")
    fct_d = nc.dram_tensor("fct", [16 * 128, 10], f32, kind="ExternalInput")
    b1_d = nc.dram_tensor("b1x4", [128, 1], f32, kind="ExternalInput")
    b2_d = nc.dram_tensor("b2", [64, 1], f32, kind="ExternalInput")
    b3_d = nc.dram_tensor("b3", [128, 1], f32, kind="ExternalInput")
    fcb_d = nc.dram_tensor("fcb", [10, 1], f32, kind="ExternalInput")
    out_d = nc.dram_tensor("out", [B_CORE, 10], f32, kind="ExternalOutput")

    def pget(t):
        a = t[:, :]
        return a.tensor, a.offset, a.ap[0][0]

    with tile.TileContext(nc) as tc:
        with tc.tile_pool(name="persist", bufs=1) as P, \
             tc.tile_pool(name="work", bufs=1) as W, \
             tc.tile_pool(name="stage", bufs=2) as SG, \
             tc.tile_pool(name="psum", bufs=4, space="PSUM") as PS, \
             tc.tile_pool(name="psumB", bufs=2, space="PSUM") as PB:

            ident = P.tile([128, 128], f32)
            make_identity(nc, ident)

            def ring():
                return PS.tile([128, 512], f32, tag="ring", name="ringt")

            def big():
                return PB.tile([128, 1024], f32, tag="big", name="bigt")

            # ---- weight load + cast (setup only) ----
            def wload(dram_t, rows, cols, nslices, tag):
                o = P.tile([rows, nslices * cols], bf16, tag=tag, name=f"w_{tag}")
                # layout: slice s occupies cols [s*cols, (s+1)*cols)
                for s in range(nslices):
                    st = SG.tile([rows, cols], f32, tag=f"wst_{tag}", name=f"wst_{tag}")
                    nc.sync.dma_start(
                        out=st[:, :],
                        in_=dram_t[s * rows:(s + 1) * rows, :])
                    nc.vector.tensor_copy(
                        out=o[:, s * cols:(s + 1) * cols], in_=st[:, :])
                return o

            S2W = wload(s2w_d, 96, 48, 32, "S2W")     # [96, 32*48]
            SEL = wload(sel_d, 48, 128, 4, "SEL")     # [48, 4*128]
            W1BD = wload(w1_d, 128, 128, 1, "W1BD")
            W2T = wload(w2_d, 96, 64, 3, "W2T")
            W3F = wload(w3f_d, 128, 128, 3, "W3F")
            W3R = wload(w3r_d, 128, 128, 3, "W3R")
            FCT = wload(fct_d, 128, 10, 16, "FCT")

            def bload(dram_t, rows, tag):
                b = P.tile([rows, 1], f32, tag=tag, name=f"b_{tag}")
                nc.sync.dma_start(out=b[:, :], in_=dram_t[:, :])
                return b
            B1 = bload(b1_d, 128, "B1")
            B2 = bload(b2_d, 64, "B2")
            B3 = bload(b3_d, 128, "B3")
            FCB = bload(fcb_d, 10, "FCB")

            # ---- persistent activations ----
            A1C = P.tile([27, L1], bf16)           # padded replica-27 tile
            A1R = P.tile([128, 32 * 8 * GB], bf16)  # compact 108-row conv1 rhs
            A2 = P.tile([96, L2], bf16)
            A3 = P.tile([128, M3 + L3 + M3], bf16)
            A4 = P.tile([128, NG * 16 * GB], bf16)
            nc.vector.memset(A1C[:, :], 0.0)
            nc.vector.memset(A1R[:, :], 0.0)
            nc.vector.memset(A2[:, :], 0.0)
            nc.vector.memset(A3[:, :], 0.0)

            a1c_t, a1c_o, a1c_p = pget(A1C)
            a1r_t, a1r_o, a1r_p = pget(A1R)
            a2_t, a2_o, a2_p = pget(A2)
            a3_t, a3_o, a3_p = pget(A3)
            a4_t, a4_o, a4_p = pget(A4)

            def chunked(n, step=512):
                base = 0
                while base < n:
                    yield base, min(step, n - base)
                    base += step

            def batch_front(bi):
                """Load 128 images; transpose; stage2; spread to S2R."""
                xr = W.tile([128, 3072], f32, tag="xr")
                nc.sync.dma_start(out=xr[:, :],
                                  in_=x_d[bi * BATCH:(bi + 1) * BATCH, :])
                XT = W.tile([96, 32 * 128], bf16, tag="XT")
                for j in range(8):   # 4 transposes -> one ring tile
                    pt = ring()
                    for t in range(4):
                        y = 4 * j + t
                        nc.tensor.transpose(
                            pt[:96, t * 128:(t + 1) * 128],
                            xr[:, 96 * y:96 * (y + 1)], ident[:, :])
                    nc.vector.tensor_copy(
                        out=XT[:, j * 512:(j + 1) * 512], in_=pt[:96, :])
                xt_t, xt_o, xt_p = pget(XT)

                # stage2: S2 [48, (jb, ib, img128)]
                S2 = W.tile([48, 8 * 8 * 128], bf16, tag="S2")
                s2_t, s2_o, s2_p = pget(S2)
                for jb in range(8):
                    for h in range(2):  # img halves of 64
                        ps = ring()
                        for ir in range(4):
                            rhs = AP(tensor=xt_t,
                                     offset=xt_o + ir * 128 + h * 64,
                                     ap=[[xt_p, 96], [512, 8], [1, 64]])
                            nc.tensor.matmul(
                                ps[:48, :], S2W[:, (jb * 4 + ir) * 48:
                                                (jb * 4 + ir + 1) * 48],
                                rhs, start=(ir == 0), stop=(ir == 3))
                        nc.vector.tensor_copy(
                            out=AP(tensor=s2_t,
                                   offset=s2_o + jb * 1024 + h * 64,
                                   ap=[[s2_p, 48], [128, 8], [1, 64]]),
                            in_=ps[:48, :])
                return S2

            def spread(S2, gp):
                """Spreader for group-pair gp: S2R [128,(r,jb,ib,img32)]."""
                s2_t, s2_o, s2_p = pget(S2)
                S2R = W.tile([128, 4 * 2048], bf16, tag="S2R")
                s2r_t, s2r_o, s2r_p = pget(S2R)
                for r in range(4):
                    for q in range(4):  # jb pairs of 2 -> N=512
                        ps = ring()
                        rhs = AP(tensor=s2_t,
                                 offset=s2_o + q * 2048 + gp * 32,
                                 ap=[[s2_p, 48], [1024, 2], [128, 8],
                                     [1, 32]])
                        nc.tensor.matmul(ps[:, :],
                                         SEL[:, r * 128:(r + 1) * 128],
                                         rhs, start=True, stop=True)
                        nc.scalar.activation(
                            out=S2R[:, r * 2048 + q * 512:
                                    r * 2048 + (q + 1) * 512],
                            in_=ps[:, :], func=ACTF.Copy, scale=1.0)
                return S2R

            def scatter(S2R, gl):
                """Scatter one group's center into A1C rows 0..3."""
                s2r_t, s2r_o, s2r_p = pget(S2R)
                for r in range(4):
                    for jc in range(4):
                        src = AP(tensor=s2r_t,
                                 offset=s2r_o + 32 * jc * s2r_p + r * 2048
                                 + (gl & 1) * 16,
                                 ap=[[s2r_p, 3], [256, 8], [32, 8], [1, 16]])
                        dst = AP(tensor=a1c_t,
                                 offset=a1c_o
                                 + ((r + 1) * X1 + jc + 1) * GB,
                                 ap=[[a1c_p, 3], [4 * GB, 8],
                                     [4 * RW1, 8], [1, GB]])
                        nc.vector.tensor_copy(out=dst, in_=src)

            def conv_body(g):
                # --- conv1 replicas: A1C rows 3..27 (4 chunked DMAs) ---
                def rep(dst_row, dst_col, src_row, src_col, nrows, length):
                    # split columns so descriptors spread over DMA engines
                    n3 = (length // 3264) or 1
                    main = n3 * 3264 if length >= 3264 else 0
                    if main:
                        nc.sync.dma_start(
                            out=AP(tensor=a1c_t,
                                   offset=a1c_o + dst_row * a1c_p + dst_col,
                                   ap=[[a1c_p, nrows], [3264, n3], [1, 3264]]),
                            in_=AP(tensor=a1c_t,
                                   offset=a1c_o + src_row * a1c_p + src_col,
                                   ap=[[a1c_p, nrows], [3264, n3], [1, 3264]]))
                    rest = length - main
                    if rest:
                        nc.sync.dma_start(
                            out=AP(tensor=a1c_t,
                                   offset=a1c_o + dst_row * a1c_p + dst_col
                                   + main,
                                   ap=[[a1c_p, nrows], [1, rest]]),
                            in_=AP(tensor=a1c_t,
                                   offset=a1c_o + src_row * a1c_p + src_col
                                   + main,
                                   ap=[[a1c_p, nrows], [1, rest]]))
                rep(3, GB, 0, 0, 3, L1 - GB)       # dx=0 block
                rep(6, 0, 0, GB, 3, L1 - GB)       # dx=2 block
                rep(9, RW1, 0, 0, 9, L1 - RW1)     # dy=0 block
                rep(18, 0, 0, RW1, 9, L1 - RW1)    # dy=2 block
                # --- stage II: compact 108-row tile (4 DMAs, one per p) ---
                for p in range(4):
                    nc.scalar.dma_start(
                        out=AP(tensor=a1r_t,
                               offset=a1r_o + 32 * p * a1r_p,
                               ap=[[a1r_p, 27], [128, 32], [1, 128]]),
                        in_=AP(tensor=a1c_t,
                               offset=a1c_o + RW1 + (8 * p + 1) * GB,
                               ap=[[a1c_p, 27], [RW1, 32], [1, 128]]))
                # --- conv1: 8 single-tap matmuls N=512, 4 wide act drains ---
                T1 = W.tile([128, 32 * 8 * GB], bf16, tag="T1")
                t1_t, t1_o, t1_p = pget(T1)
                for base, n in chunked(32 * 8 * GB, 1024):
                    ps = big()
                    for sub in range(0, n, 512):
                        rhs = AP(tensor=a1r_t, offset=a1r_o + base + sub,
                                 ap=[[a1r_p, 128], [1, 512]])
                        nc.tensor.matmul(ps[:, sub:sub + 512], W1BD[:, :],
                                         rhs, start=True, stop=True)
                    nc.scalar.activation(out=T1[:, base:base + n],
                                         in_=ps[:, :n], func=ACTF.Relu,
                                         bias=B1[:, :], scale=1.0)
                # --- pool1 (rows (p,c), cols (y32, k8, b)) ---
                X1p = W.tile([128, 32 * 4 * GB], bf16, tag="X1p")
                x1_t, x1_o, x1_p = pget(X1p)
                nc.vector.tensor_tensor(
                    out=X1p[:, :],
                    in0=AP(tensor=t1_t, offset=t1_o,
                           ap=[[t1_p, 128], [128, 32], [32, 4], [1, GB]]),
                    in1=AP(tensor=t1_t, offset=t1_o + GB,
                           ap=[[t1_p, 128], [128, 32], [32, 4], [1, GB]]),
                    op=OP.max)
                X1q = W.tile([128, 16 * 4 * GB], bf16, tag="X1q")
                x1q_t, x1q_o, x1q_p = pget(X1q)
                nc.vector.tensor_tensor(
                    out=X1q[:, :],
                    in0=AP(tensor=x1_t, offset=x1_o,
                           ap=[[x1_p, 128], [128, 16], [1, 64]]),
                    in1=AP(tensor=x1_t, offset=x1_o + 64,
                           ap=[[x1_p, 128], [128, 16], [1, 64]]),
                    op=OP.max)
                # regroup 4 x-blocks into A2 center rows 32..64
                for p in range(4):
                    nc.vector.tensor_copy(
                        out=AP(tensor=a2_t,
                               offset=a2_o + 32 * a2_p + RW2
                               + (4 * p + 1) * GB,
                               ap=[[a2_p, 32], [RW2, 16], [1, 64]]),
                        in_=AP(tensor=x1q_t,
                               offset=x1q_o + 32 * p * x1q_p,
                               ap=[[x1q_p, 32], [64, 16], [1, 64]]))
                # --- A2 dx replicas (2 chunked DMAs over L2-GB=5168) ---
                nc.scalar.dma_start(
                    out=AP(tensor=a2_t, offset=a2_o + GB,
                           ap=[[a2_p, 32], [1292, 4], [1, 1292]]),
                    in_=AP(tensor=a2_t, offset=a2_o + 32 * a2_p,
                           ap=[[a2_p, 32], [1292, 4], [1, 1292]]))
                nc.scalar.dma_start(
                    out=AP(tensor=a2_t, offset=a2_o + 64 * a2_p,
                           ap=[[a2_p, 32], [1292, 4], [1, 1292]]),
                    in_=AP(tensor=a2_t, offset=a2_o + 32 * a2_p + GB,
                           ap=[[a2_p, 32], [1292, 4], [1, 1292]]))
                # --- conv2: 3 dy taps, cols [RW2, 17*RW2) ---
                T2 = W.tile([64, 16 * RW2], bf16, tag="T2")
                t2_t, t2_o, t2_p = pget(T2)
                for base, n in chunked(16 * RW2, 1024):
                    ps = big()
                    for sub in range(0, n, 512):
                        ns = min(512, n - sub)
                        for dy in range(3):
                            rhs = AP(tensor=a2_t,
                                     offset=a2_o + base + sub + dy * RW2,
                                     ap=[[a2_p, 96], [1, ns]])
                            nc.tensor.matmul(ps[:64, sub:sub + ns],
                                             W2T[:, dy * 64:(dy + 1) * 64],
                                             rhs, start=(dy == 0),
                                             stop=(dy == 2))
                    nc.scalar.activation(out=T2[:, base:base + n],
                                         in_=ps[:64, :n], func=ACTF.Relu,
                                         bias=B2[:, :], scale=1.0)
                # --- pool2 -> A3 center rows 64..128 ---
                X2p = W.tile([64, 16 * 8 * GB], bf16, tag="X2p")
                x2_t, x2_o, x2_p = pget(X2p)
                nc.vector.tensor_tensor(
                    out=X2p[:, :],
                    in0=AP(tensor=t2_t, offset=t2_o + GB,
                           ap=[[t2_p, 64], [RW2, 16], [32, 8], [1, GB]]),
                    in1=AP(tensor=t2_t, offset=t2_o + 2 * GB,
                           ap=[[t2_p, 64], [RW2, 16], [32, 8], [1, GB]]),
                    op=OP.max)
                nc.vector.tensor_tensor(
                    out=AP(tensor=a3_t,
                           offset=a3_o + 64 * a3_p + M3 + RW3 + GB,
                           ap=[[a3_p, 64], [RW3, 8], [1, 128]]),
                    in0=AP(tensor=x2_t, offset=x2_o,
                           ap=[[x2_p, 64], [256, 8], [1, 128]]),
                    in1=AP(tensor=x2_t, offset=x2_o + 128,
                           ap=[[x2_p, 64], [256, 8], [1, 128]]),
                    op=OP.max)
                # --- A3 dx replica: rows 0..64 = center shifted +GB ---
                nc.vector.tensor_copy(
                    out=AP(tensor=a3_t, offset=a3_o + GB,
                           ap=[[a3_p, 64], [1, M3 + L3 + M3 - GB]]),
                    in_=AP(tensor=a3_t, offset=a3_o + 64 * a3_p,
                           ap=[[a3_p, 64], [1, M3 + L3 + M3 - GB]]))
                # --- conv3: (K=128 fold + K=64 rem) x 3 dy ---
                T3 = W.tile([128, 8 * RW3], bf16, tag="T3")
                t3_t, t3_o, t3_p = pget(T3)
                for base, n in chunked(8 * RW3, 1024):
                    ps = big()
                    for sub in range(0, n, 512):
                        ns = min(512, n - sub)
                        for dy in range(3):
                            rhs = AP(tensor=a3_t,
                                     offset=a3_o + M3 + base + sub
                                     + dy * RW3,
                                     ap=[[a3_p, 128], [1, ns]])
                            nc.tensor.matmul(ps[:, sub:sub + ns],
                                             W3F[:, dy * 128:(dy + 1) * 128],
                                             rhs, start=(dy == 0), stop=False)
                            rhs2 = AP(tensor=a3_t,
                                      offset=a3_o + 64 * a3_p + M3 + base
                                      + sub + dy * RW3 + GB,
                                      ap=[[a3_p, 64], [1, ns]])
                            nc.tensor.matmul(ps[:, sub:sub + ns],
                                             W3R[64:128,
                                                 dy * 128:(dy + 1) * 128],
                                             rhs2, start=False,
                                             stop=(dy == 2))
                    nc.scalar.activation(out=T3[:, base:base + n],
                                         in_=ps[:, :n], func=ACTF.Relu,
                                         bias=B3[:, :], scale=1.0)
                # --- pool3 -> A4 ---
                X3p = W.tile([128, 8 * 4 * GB], bf16, tag="X3p")
                x3_t, x3_o, x3_p = pget(X3p)
                nc.vector.tensor_tensor(
                    out=X3p[:, :],
                    in0=AP(tensor=t3_t, offset=t3_o + GB,
                           ap=[[t3_p, 128], [RW3, 8], [32, 4], [1, GB]]),
                    in1=AP(tensor=t3_t, offset=t3_o + 2 * GB,
                           ap=[[t3_p, 128], [RW3, 8], [32, 4], [1, GB]]),
                    op=OP.max)
                nc.vector.tensor_tensor(
                    out=A4[:, g * 256:(g + 1) * 256],
                    in0=AP(tensor=x3_t, offset=x3_o,
                           ap=[[x3_p, 128], [128, 4], [1, 64]]),
                    in1=AP(tensor=x3_t, offset=x3_o + 64,
                           ap=[[x3_p, 128], [128, 4], [1, 64]]),
                    op=OP.max)

            for bi in range(NB):
                S2 = batch_front(bi)
                for gp in range(4):
                    S2R = spread(S2, gp)
                    for gh in range(2):
                        gl = gp * 2 + gh
                        g = bi * GPB + gl
                        scatter(S2R, gl)
                        conv_body(g)

            # ---------------- FC + log_softmax ----------------
            for bq in range(2):  # 512 images each
                ps = ring()
                for k in range(16):
                    rhs = AP(tensor=a4_t,
                             offset=a4_o + bq * 32 * 256 + k * GB,
                             ap=[[a4_p, 128], [256, 32], [1, GB]])
                    nc.tensor.matmul(ps[:10, :], FCT[:, k * 10:(k + 1) * 10],
                                     rhs, start=(k == 0), stop=(k == 15))
                lg = W.tile([10, 512], f32, tag="lgs")
                nc.vector.tensor_scalar(lg[:, :], ps[:10, :], FCB[:, :],
                                        None, OP.add)
                for h in range(4):  # 128 images per transpose
                    pt = ring()
                    nc.tensor.transpose(pt[:, :10],
                                        lg[:, h * 128:(h + 1) * 128],
                                        ident[:10, :10])
                    z = W.tile([128, 10], f32, tag="z")
                    nc.vector.tensor_copy(out=z[:, :], in_=pt[:, :10])
                    m = W.tile([128, 1], f32, tag="m")
                    nc.vector.tensor_reduce(out=m[:, :], in_=z[:, :],
                                            axis=mybir.AxisListType.X,
                                            op=OP.max, negate=True)
                    e = W.tile([128, 10], f32, tag="e")
                    nc.scalar.activation(out=e[:, :], in_=z[:, :],
                                         func=ACTF.Exp, bias=m[:, :],
                                         scale=1.0)
                    s = W.tile([128, 1], f32, tag="s")
                    nc.vector.tensor_reduce(out=s[:, :], in_=e[:, :],
                                            axis=mybir.AxisListType.X,
                                            op=OP.add)
                    ls = W.tile([128, 1], f32, tag="ls")
                    nc.scalar.activation(out=ls[:, :], in_=s[:, :],
                                         func=ACTF.Ln)
                    nc.vector.tensor_scalar(ls[:, :], ls[:, :], m[:, :],
                                            None, OP.subtract)
                    o = W.tile([128, 10], f32, tag="o")
                    nc.vector.tensor_scalar(o[:, :], z[:, :], ls[:, :],
                                            None, OP.subtract)
                    bc = bq * 4 + h
                    nc.sync.dma_start(out=out_d[bc * 128:(bc + 1) * 128, :],
                                      in_=o[:, :])

    nc.compile()
    return nc


_NC_CACHE = {}
LAST_RESULT = None
LAST_USED_BASS = False


def _run_bass(x, W_lin, conv1_w, conv1_b, conv2_w, conv2_b, conv3_w, conv3_b,
              fc_w, fc_b, stage, trace=False):
    global LAST_RESULT
    from concourse.bass_utils import run_bass_kernel_spmd
    if int(stage) != 2:
        raise NotImplementedError("bass path only implements stage=2")
    if "nc" not in _NC_CACHE:
        _NC_CACHE["nc"] = _build_bass()
    nc = _NC_CACHE["nc"]
    xs = np.ascontiguousarray(x, dtype=np.float32).reshape(N_CORES, B_CORE, 3072)
    common = _pack_weights(W_lin, conv1_w, conv2_w, conv3_w, fc_w,
                           conv1_b, conv2_b, conv3_b, fc_b)
    in_maps = [dict(common, x=xs[i]) for i in range(N_CORES)]
    res = run_bass_kernel_spmd(nc, in_maps, core_ids=list(range(N_CORES)),
                               trace=trace)
    LAST_RESULT = res
    return np.concatenate([r["out"] for r in res.results], axis=0)


def kernel(**inputs) -> np.ndarray:
    import os
    global LAST_USED_BASS
    stage = inputs.get("stage", 2)
    args = {k: np.asarray(v) for k, v in inputs.items() if k != "stage"}
    trace = os.environ.get("KERNEL_TRACE", "") == "1"
    try:
        out = _run_bass(stage=stage, trace=trace, **args)
        LAST_USED_BASS = True
        return out
    except Exception as e:
        import traceback, sys
        traceback.print_exc()
        print(f"[kernel] Bass path failed ({type(e).__name__}); "
              "falling back to JAX host implementation", file=sys.stderr)
        LAST_USED_BASS = False
        return _jax_reference(stage=stage, **args)
